# revision 21
# baseline (speedup 1.0000x reference)
"""Causal self-attention (B=4, T=2048, C=2048, H=16, rope) on 8 trn2 cores.

Sharding: tensor-parallel over heads — 2 heads per core. Each core computes
q/k/v projections for its head slice from the full x, runs causal attention,
and produces a partial output projection y_c = attn_c @ wo[:, slice].T.
The host sums the 8 partial y tensors (row-parallel linear unshard).

All matmul operands are bf16 (PSUM accumulation stays fp32): bf16 runs
1 cycle/row on the PE vs ~1.5 for fp32r, enables fast weight load, and
halves DMA traffic. Phase B pairs two 128-token S blocks into one 2-bank
PSUM tile so a single ScalarE ACTIVATE exps both (the 352-cycle fixed
ACTIVATE overhead otherwise makes exp the phase-B rate limiter), and is
software-pipelined (S of pair p+1 before AV/RS of pair p) so the PE
streams. Diagonal 128-col sub-blocks beyond the causal frontier are
skipped. Phase C drains PSUM in 2-bank pairs alternating DVE/ScalarE.
"""

import numpy as np
import ml_dtypes

import concourse.bass as bass
import concourse.mybir as mybir
import concourse.tile as tile
from concourse.vector_clock import ScopedClock
from concourse.bass_utils import run_bass_kernel_spmd

# ---------------------------------------------------------------- tile patch
# The pinned walrus codegen accepts at most ONE sync-wait per hardware
# instruction; Tile attaches several. Split extras onto same-engine NOPs.

_MAX_WAITS = 1
_orig_add_instruction = tile.TileContext._add_instruction


def _split_add_instruction(self, inst):
    si = getattr(inst, "sync_info", None)
    if si is not None and len(si.on_wait) > _MAX_WAITS:
        waits = list(si.on_wait)
        extras, keep = waits[:-_MAX_WAITS], waits[-_MAX_WAITS:]
        inst.sync_info = mybir.SyncInfo(on_wait=keep, on_update=list(si.on_update))
        for i in range(0, len(extras), _MAX_WAITS):
            nop = mybir.InstNoOp(
                name=f"{inst.name}-ws{i}",
                sync_info=mybir.SyncInfo(on_wait=extras[i : i + _MAX_WAITS], on_update=[]),
                engine=inst.engine,
                bass_nofuse=True,
            )
            _orig_add_instruction(self, nop)
    _orig_add_instruction(self, inst)


def _patched_drain_and_barrier(self, tick_clock, wait_clock):
    nc = self.nc
    drain_inst = nc.sync.drain()
    wait_clock.add_sem_waits(drain_inst.ins, ScopedClock({None: tick_clock.global_clock}))
    si = drain_inst.ins.sync_info
    if si is not None and len(si.on_wait) > 1:
        waits = list(si.on_wait)
        drain_inst.ins.sync_info = mybir.SyncInfo(on_wait=waits[:1], on_update=list(si.on_update))
        for w in waits[1:]:
            extra = nc.sync.drain()
            extra.ins.sync_info = mybir.SyncInfo(on_wait=[w], on_update=[])
    nc.all_engine_barrier()
    assert self.sems is not None
    popped = nc._tile_sem_poison_stack.pop()
    assert popped is self._sem_poison
    nc.clear_and_free_semaphores(list(self.sems.allocated().values()))
    nc.all_engine_barrier()


tile.TileContext._add_instruction = _split_add_instruction
tile.TileContext._drain_and_barrier = _patched_drain_and_barrier

# ---------------------------------------------------------------- constants

B, T, C, H, D = 4, 2048, 2048, 16, 128
N_CORES = 8
HPC = H // N_CORES        # heads per core = 2
M = HPC * D               # per-core projection width = 256
BT = B * T
KT = C // 128             # 16 k-subtiles
SCALE = 1.0 / float(np.sqrt(D))
NEG = -30000.0            # pre-scale additive mask value; exp(scale*(s+NEG)) == 0

F32 = mybir.dt.float32
BF16 = mybir.dt.bfloat16

ALU = mybir.AluOpType
AF = mybir.ActivationFunctionType


def build_kernel():
    nc = bass.Bass("TRN2", target_bir_lowering=False, debug=False)

    xT = nc.dram_tensor("xT", [BT // 512, 128, KT, 512], BF16, kind="ExternalInput").ap()
    wqT = nc.dram_tensor("wqT", [C, M], BF16, kind="ExternalInput").ap()
    wkT = nc.dram_tensor("wkT", [C, M], BF16, kind="ExternalInput").ap()
    wvT = nc.dram_tensor("wvT", [C, M], BF16, kind="ExternalInput").ap()
    woT = nc.dram_tensor("woT", [M, C], BF16, kind="ExternalInput").ap()
    cosT = nc.dram_tensor("cosT", [D, T], F32, kind="ExternalInput").ap()
    sinT = nc.dram_tensor("sinT", [D, T], F32, kind="ExternalInput").ap()
    # causal mask is built ON the PE: sp_diag = triuU^T @ (NEG*I) (+= S later)
    triuU = nc.dram_tensor("triuU", [128, 128], BF16, kind="ExternalInput").ap()
    negI = nc.dram_tensor("negI", [128, 128], BF16, kind="ExternalInput").ap()
    ones = nc.dram_tensor("ones", [128, 128], BF16, kind="ExternalInput").ap()
    y = nc.dram_tensor("y", [BT // 128, 128, C], BF16, kind="ExternalOutput").ap()

    with tile.TileContext(nc) as tc:
        with (
            tc.tile_pool(name="const", bufs=1) as constp,
            tc.tile_pool(name="cs", bufs=2) as csp,
            tc.tile_pool(name="xpool", bufs=2) as xpool,
            tc.tile_pool(name="qpool", bufs=2) as qpool,
            tc.tile_pool(name="kvpool", bufs=1) as kvpool,
            tc.tile_pool(name="attnpool", bufs=2) as attnpool,
            tc.tile_pool(name="ptpool", bufs=4) as ptpool,
            tc.tile_pool(name="tmp", bufs=3) as tmpp,
            tc.tile_pool(name="ystg", bufs=4) as ystg,
            tc.tile_pool(name="ps_big", bufs=2, space="PSUM") as ps_big,
            tc.tile_pool(name="ps_av", bufs=2, space="PSUM") as ps_av,
            tc.tile_pool(name="ps_rs", bufs=2, space="PSUM") as ps_rs,
        ):
            # ---- resident constants. DMA issue order is chosen so the very
            # first q matmuls can start ~3us in: x half, wq half, ...; wo
            # (needed only in phase C) goes last.
            wq_sb = constp.tile([128, KT, M], BF16, tag="wq")
            wk_sb = constp.tile([128, KT, M], BF16, tag="wk")
            wv_sb = constp.tile([128, KT, M], BF16, tag="wv")
            wo_sb = constp.tile([128, HPC, C], BF16, tag="wo")
            triu_sb = constp.tile([128, 128], BF16, tag="triu")
            negi_sb = constp.tile([128, 128], BF16, tag="negi")
            ones_sb = constp.tile([128, 128], BF16, tag="ones")
            x_first = xpool.tile([128, KT, 512], BF16, tag="x", name="x_t")
            KH = KT // 4
            wqr = wqT.rearrange("(ko p) m -> p ko m", p=128)
            wkr = wkT.rearrange("(ko p) m -> p ko m", p=128)
            wvr = wvT.rearrange("(ko p) m -> p ko m", p=128)
            for kq in range(4):  # interleave so the first q matmuls start early
                sl = slice(kq * KH, (kq + 1) * KH)
                nc.sync.dma_start(x_first[:, sl], xT[0, :, sl])
                nc.sync.dma_start(wq_sb[:, sl], wqr[:, sl])
            cos_first = csp.tile([D, 512], F32, tag="cos", name="cos_t")
            sin_first = csp.tile([D, 512], F32, tag="sin", name="sin_t")
            nc.sync.dma_start(cos_first[:], cosT[:, 0:512])
            nc.sync.dma_start(sin_first[:], sinT[:, 0:512])
            nc.sync.dma_start(wk_sb[:, :KH], wkr[:, :KH])
            nc.sync.dma_start(wk_sb[:, KH:], wkr[:, KH:])
            nc.sync.dma_start(wv_sb[:, :KH], wvr[:, :KH])
            nc.sync.dma_start(wv_sb[:, KH:], wvr[:, KH:])
            nc.sync.dma_start(triu_sb[:], triuU[:])
            nc.sync.dma_start(negi_sb[:], negI[:])
            nc.sync.dma_start(ones_sb[:], ones[:])
            nc.sync.dma_start(wo_sb[:], woT.rearrange("(mh p) j -> p mh j", p=128))

            wqk = [wq_sb, wq_sb, wk_sb, wk_sb]
            pending_c = None  # phase C lags one block so divides can finish

            def emit_phase_c(b, a, attn_sb):
                # Lagged phase C borrows the av/rs PSUM banks (idle between
                # blocks): 16 single-bank tiles rotating through 4 banks, so
                # drains have ~4 fills of slack. Drains alternate engines.
                for nt in range(4):
                    yt = ystg.tile([128, C], BF16, tag="y", name="yt")
                    for jb in range(4):
                        pool = ps_av if jb % 2 == 0 else ps_rs
                        tg = "av" if jb % 2 == 0 else "rs"
                        yp = pool.tile([128, 512], F32, tag=tg, name="yp")
                        for mh in range(HPC):
                            nc.tensor.matmul(
                                yp[:],
                                attn_sb[:, mh, nt * 128 : (nt + 1) * 128],
                                wo_sb[:, mh, jb * 512 : (jb + 1) * 512],
                                start=(mh == 0),
                                stop=(mh == HPC - 1),
                            )
                        dstc = yt[:, jb * 512 : (jb + 1) * 512]
                        if (nt + jb) % 2 == 0:
                            nc.vector.tensor_copy(dstc, yp[:])
                        else:
                            nc.scalar.copy(dstc, yp[:])
                    rt = (b * T + a * 512 + nt * 128) // 128
                    nc.sync.dma_start(y[rt], yt[:])

            for b in range(B):
                # k/v for the whole sequence of this batch accumulate here
                k_sb = kvpool.tile([D, HPC, T], BF16, tag="k")
                v_sb = kvpool.tile([128, T // 128, M], BF16, tag="v")

                for a in range(4):  # 512-token block (QKV -> attn -> proj)
                    t0 = a * 512
                    q_sb = qpool.tile([D, HPC, 512], BF16, tag="q")
                    attn_sb = attnpool.tile([D, HPC, 512], BF16, tag="attn")

                    # -------- phase A: qkv + rope for tokens [t0, t0+512)
                    if b == 0 and a == 0:
                        x_t, cos_t, sin_t = x_first, cos_first, sin_first
                        nxt = {}
                    else:
                        x_t, cos_t, sin_t = nxt[(b, a)]
                    # prefetch next block's x/cos/sin now: issued ahead of
                    # this block's y DMAs on the sync queue, so the transfer
                    # runs during phase B instead of stalling phase A(a+1)
                    blk = b * 4 + a + 1
                    if blk < B * 4:
                        xn = xpool.tile([128, KT, 512], BF16, tag="x", name="x_t")
                        nc.sync.dma_start(xn[:], xT[blk])
                        cn = csp.tile([D, 512], F32, tag="cos", name="cos_t")
                        sn = csp.tile([D, 512], F32, tag="sin", name="sin_t")
                        tn = (blk % 4) * 512
                        nc.sync.dma_start(cn[:], cosT[:, tn : tn + 512])
                        nc.sync.dma_start(sn[:], sinT[:, tn : tn + 512])
                        nxt[(blk // 4, blk % 4)] = (xn, cn, sn)

                    for m in range(4):  # q0 q1 k0 k1
                        h = m % 2
                        psf = ps_big.tile([128, 1024], F32, tag="big", name="mm")
                        ps = psf[:, :512]
                        w_sb = wqk[m]
                        for kt in range(KT):
                            nc.tensor.matmul(
                                ps,
                                w_sb[:, kt, h * D : (h + 1) * D],
                                x_t[:, kt, :],
                                start=(kt == 0),
                                stop=(kt == KT - 1),
                            )
                        rot = tmpp.tile([128, 512], F32, tag="tmp", name="rot")
                        t1 = tmpp.tile([128, 512], F32, tag="tmp", name="t1")
                        nc.vector.tensor_scalar_mul(rot[0:64, :], psf[64:128, :512], -1.0)
                        nc.vector.tensor_copy(rot[64:128, :], psf[0:64, :512])
                        nc.vector.tensor_tensor(t1[:], ps, cos_t[:], ALU.mult)
                        nc.vector.tensor_tensor(rot[:], rot[:], sin_t[:], ALU.mult)
                        dst = q_sb if m < 2 else k_sb
                        col = 0 if m < 2 else t0
                        nc.vector.tensor_tensor(
                            dst[:, h, col : col + 512], t1[:], rot[:], ALU.add
                        )

                    for nt in range(4):  # v in [t, d] layout directly
                        vp_full = ps_big.tile([128, 1024], F32, tag="big", name="vp")
                        vp = vp_full[:, :M]
                        for kt in range(KT):
                            nc.tensor.matmul(
                                vp,
                                x_t[:, kt, nt * 128 : (nt + 1) * 128],
                                wv_sb[:, kt, :],
                                start=(kt == 0),
                                stop=(kt == KT - 1),
                            )
                        nc.vector.tensor_copy(v_sb[:, a * 4 + nt, :], vp)

                    # -------- lagged phase C of the previous block: its PE
                    # work runs here while the previous block's softmax
                    # divides finish on ScalarE/DVE.
                    if pending_c is not None:
                        emit_phase_c(*pending_c)
                    pending_c = (b, a, attn_sb)

                    # -------- phase B: attention for i-block a, both heads.
                    # Two 128-token j-blocks share one 2-bank PSUM tile; one
                    # ACTIVATE exps both halves. S/exp of pair p+1 is issued
                    # before AV/RS of pair p so the PE streams while ScalarE
                    # exps. Diagonal sub-blocks are trimmed to i >= 128*kk.
                    njt = 4 * a + 4

                    def emit_s_pair(p, h):
                        spp = ps_big.tile([128, 1024], F32, tag="big", name="spp")
                        offs = [128 * (2 * p + t - 4 * a) if 2 * p + t >= 4 * a else 0
                                for t in (0, 1)]
                        diag = 2 * p >= 4 * a
                        for t in (0, 1):
                            jt = 2 * p + t
                            if diag:
                                # seed the diagonal square with the additive
                                # causal mask ON the PE (start clears the
                                # bank); S accumulates on top. No DVE hop.
                                col = 512 * t + offs[t]
                                nc.tensor.matmul(
                                    spp[:, col : col + 128],
                                    triu_sb[:],
                                    negi_sb[:],
                                    start=True,
                                    stop=False,
                                )
                            nc.tensor.matmul(
                                spp[:, 512 * t + offs[t] : 512 * (t + 1)],
                                k_sb[:, h, jt * 128 : (jt + 1) * 128],
                                q_sb[:, h, offs[t] :],
                                start=not diag,
                                stop=True,
                            )
                        pt = ptpool.tile([128, 1024], BF16, tag="pt", name="pt")
                        lo = offs[0]
                        nc.scalar.activation(pt[:, lo:], spp[:, lo:], AF.Exp, scale=SCALE)
                        return p, offs, pt

                    # interleave the two heads' pairs so the exp latency of
                    # one head is covered by the other head's matmuls — one
                    # pipeline fill per block instead of two.
                    av = {hh: ps_av.tile([128, 512], F32, tag="av", name="av")
                          for hh in range(HPC)}
                    rsp = {hh: ps_rs.tile([128, 512], F32, tag="rs", name="rsp")
                           for hh in range(HPC)}

                    def emit_avrs(p, offs, pt, hh):
                        for t in (0, 1):
                            jt = 2 * p + t
                            off = offs[t]
                            mv = pt[:, 512 * t + off : 512 * (t + 1)]
                            nc.tensor.matmul(
                                av[hh][:, off:],
                                v_sb[:, jt, hh * D : (hh + 1) * D],
                                mv,
                                start=(jt == 0),
                                stop=(jt == njt - 1),
                            )
                            nc.tensor.matmul(
                                rsp[hh][:, off:],
                                ones_sb[:],
                                mv,
                                start=(jt == 0),
                                stop=(jt == njt - 1),
                            )

                    seq = [(p, hh) for p in range(njt // 2) for hh in range(HPC)]
                    prev = None
                    for p, hh in seq:
                        cur = emit_s_pair(p, hh) + (hh,)
                        if prev is not None:
                            emit_avrs(*prev)
                        prev = cur
                    emit_avrs(*prev)

                    for hh in range(HPC):
                        # 1/rowsum as exp(-ln(x)) on ScalarE — DVE's iterative
                        # reciprocal is ~3.4us; exp+ln share one ACT table set.
                        lrs = tmpp.tile([128, 512], F32, tag="rcp", name="lrs", bufs=2)
                        rec = tmpp.tile([128, 512], F32, tag="rcp", name="rec", bufs=2)
                        nc.scalar.activation(lrs[:], rsp[hh][:], AF.Ln)
                        nc.scalar.activation(rec[:], lrs[:], AF.Exp, scale=-1.0)
                        nc.vector.tensor_tensor(attn_sb[:, hh, :], av[hh][:], rec[:], ALU.mult)

            assert pending_c is not None
            emit_phase_c(*pending_c)
    return nc


_NC_CACHE = {}


def _get_nc(**kw):
    key = tuple(sorted(kw.items()))
    if key not in _NC_CACHE:
        _NC_CACHE[key] = build_kernel(**kw)
    return _NC_CACHE[key]


def make_inputs(x, freqs_cos, freqs_sin, wq, wk, wv, wo):
    """Host-side shard prep: returns in_maps for the 8 cores."""
    bf16 = ml_dtypes.bfloat16
    x = np.asarray(x, dtype=np.float32)
    # blocked xT: [BT/512 blocks, 128 p, KT, 512 tokens], contiguous per block
    xT = np.ascontiguousarray(
        x.reshape(BT // 512, 512, KT, 128).transpose(0, 3, 2, 1)
    ).astype(bf16)
    cosT = np.ascontiguousarray(np.asarray(freqs_cos, np.float32).T)
    sinT = np.ascontiguousarray(np.asarray(freqs_sin, np.float32).T)
    p = np.arange(128)[:, None]
    g = np.arange(128)[None, :]
    # PE-side mask factors: (triuU^T @ negI)[p,i] == NEG iff p > i
    triuU = (g > p).astype(np.float32).astype(bf16)          # U[c,p]=1 iff p>c
    negI = (NEG * np.eye(128, dtype=np.float32)).astype(bf16)
    ones = np.ones((128, 128), bf16)
    in_maps = []
    for c in range(N_CORES):
        sl = slice(c * M, (c + 1) * M)
        in_maps.append(
            {
                "xT": xT,
                "wqT": np.ascontiguousarray(np.asarray(wq, np.float32)[sl, :].T).astype(bf16),
                "wkT": np.ascontiguousarray(np.asarray(wk, np.float32)[sl, :].T).astype(bf16),
                "wvT": np.ascontiguousarray(np.asarray(wv, np.float32)[sl, :].T).astype(bf16),
                "woT": np.ascontiguousarray(np.asarray(wo, np.float32)[:, sl].T).astype(bf16),
                "cosT": cosT,
                "sinT": sinT,
                "triuU": triuU,
                "negI": negI,
                "ones": ones,
            }
        )
    return in_maps


def kernel(x, freqs_cos, freqs_sin, wq, wk, wv, wo):
    nc = _get_nc()
    in_maps = make_inputs(x, freqs_cos, freqs_sin, wq, wk, wv, wo)
    res = run_bass_kernel_spmd(nc, in_maps, list(range(N_CORES)))
    out = np.zeros((BT // 128, 128, C), np.float32)
    for r in res.results:
        out += r["y"].astype(np.float32)
    return out.reshape(B, T, C)
